# revision 5
# baseline (speedup 1.0000x reference)
"""Trainium2 Bass kernel for BestRQ VQ codebook lookup (fp32r single-pass).

Pipeline (per NeuronCore, data-parallel over batch — core i takes batch row i):
  x (2048,512) --LayerNorm (DVE)--> xn --PE transpose--> xnT (d-major)
  t^T = projW^T @ xn^T  as a bf16 3-pass hi/lo matmul (hi@hi + hi@lo +
    lo@hi, ~fp32-exact: bf16 residuals keep fp32's exponent range so no
    scaling is needed; 3 cyc/row vs 4 for plain fp32)
  t^T stored as fp32r (tf32-like e8m11, round-to-nearest)
  codebook streamed in 1024-column chunks, DMA'd straight into fp32r tiles
  score = t.c + (512 - 0.5*||c||^2), computed per (token-tile, chunk) as a
    single-pass fp32r accumulation: 8 K=128 matmuls + one K=2 matmul of
    [ones;ones] @ [bias_hi;bias_lo].  fp32r runs at 1 cycle/row on the PE
    for moving dims >= 256 (vs 4 for fp32, 3 passes for fp16 hi/lo), so the
    cross matmul runs at full PE rate in one pass.
  argmax: DVE max/max_index read the PSUM accumulator directly (no SBUF
    copy); per-chunk winners combined over 8 chunks at the end.

Numerics notes (measured on HW):
  - fp32r operands are rounded to ~11 mantissa bits (round-to-nearest),
    both when DMA'd raw and when produced by Act copies.
  - The distance bias -0.5*||c||^2 is centered (+512, argmax-invariant) and
    split hi/lo across the two K-rows of the bias matmul: a single tf32 row
    would quantize the +-150-range bias at ~0.1 and flip argmins.
  - The projection must be (near-)exact: computing it in fp32r compounds
    operand rounding into t and doubles the label-flip count.
  Net: 3/16384 label flips vs the fp32 reference (rel err 1.24e-2 < 2e-2).
"""

import numpy as np

import concourse.bacc as bacc
import concourse.bass as bass
import concourse.mybir as mybir
import concourse.tile as tile
from concourse.bass_utils import run_bass_kernel_spmd
from concourse.masks import make_identity

B, L, D, H, C = 8, 2048, 512, 1024, 8192
LN_EPS = 1e-5
N_CORES = 8

TT = L // 128      # 16 token tiles
CW = 1024          # codebook chunk width
CCH = C // CW      # 8 codebook chunks
NG = CW // 512     # 512-wide matmul groups per chunk
HT = H // 128      # 8 h tiles
DT = D // 128      # 4 d tiles
TOKC = L // 512    # 4 token chunks (projection)

F32 = mybir.dt.float32
F32R = mybir.dt.float32r
BF16 = mybir.dt.bfloat16
I32 = mybir.dt.int32
U32 = mybir.dt.uint32


def build_nc(repeat=1):
    nc = bacc.Bacc("TRN2", target_bir_lowering=False, debug=False)

    d_x = nc.dram_tensor("x", (L, D), F32, kind="ExternalInput")
    d_pw = nc.dram_tensor("pw", (H, D), F32, kind="ExternalInput")
    d_lnw = nc.dram_tensor("lnw", (D,), F32, kind="ExternalInput")
    d_lnb = nc.dram_tensor("lnb", (D,), F32, kind="ExternalInput")
    d_cb = nc.dram_tensor("cb", (H, C), F32R, kind="ExternalInput")
    d_cbt = nc.dram_tensor("cbt", (C, H), F32, kind="ExternalInput")
    d_lab = nc.dram_tensor("labels", (128, TT), I32, kind="ExternalOutput")

    with tile.TileContext(nc) as tc:
      for _rep in range(repeat):
        with tc.tile_pool(name="consts", bufs=1) as consts, \
             tc.tile_pool(name="persist", bufs=1) as persist, \
             tc.tile_pool(name="dram", bufs=1, space="DRAM") as dram:
            scratch = dram.tile([C], F32)

            # ---------- constants ----------
            ident = consts.tile([128, 128], F32)
            make_identity(nc, ident)
            eps_t = consts.tile([128, 1], F32)
            nc.vector.memset(eps_t, LN_EPS)
            lnw_bc = consts.tile([128, D], F32)
            nc.sync.dma_start(
                out=lnw_bc,
                in_=bass.AP(tensor=d_lnw, offset=0, ap=[[0, 128], [1, D]]))
            lnb_bc = consts.tile([128, D], F32)
            nc.sync.dma_start(
                out=lnb_bc,
                in_=bass.AP(tensor=d_lnb, offset=0, ap=[[0, 128], [1, D]]))
            chunk_off = consts.tile([128, CCH], F32)
            for j in range(CCH):
                nc.vector.memset(chunk_off[:, j:j + 1], float(CW * j))
            ones_f = consts.tile([2, 128], F32)
            nc.vector.memset(ones_f, 1.0)
            ones2 = consts.tile([2, 128], F32R)
            nc.scalar.copy(out=ones2, in_=ones_f)

            # persistent t^T in fp32r: (h, tok) layout
            th = [persist.tile([128, L], F32R, name=f"th{h}", tag=f"th{h}")
                  for h in range(HT)]

            # ---------- phase A: LN + transposes + projection ----------
            with tc.tile_pool(name="phA", bufs=1) as phA, \
                 tc.tile_pool(name="ldtmp", bufs=3) as ldtmp, \
                 tc.tile_pool(name="psA", bufs=2, space="PSUM") as psA, \
                 tc.tile_pool(name="psTr", bufs=2, space="PSUM") as psTr:

                # proj weight: load (h,d), PE-transpose to (d,h)
                pwT = [phA.tile([128, H], F32, name=f"pwT{d}", tag=f"pwT{d}")
                       for d in range(DT)]
                for h in range(HT):
                    pw_t = ldtmp.tile([128, D], F32, tag="pw_t")
                    nc.sync.dma_start(out=pw_t, in_=d_pw[h * 128:(h + 1) * 128, :])
                    for d in range(DT):
                        ps_tr = psTr.tile([128, 128], F32, tag="ps_tr")
                        nc.tensor.transpose(ps_tr, pw_t[:, d * 128:(d + 1) * 128],
                                            ident)
                        nc.scalar.copy(out=pwT[d][:, h * 128:(h + 1) * 128],
                                       in_=ps_tr)

                # bf16 hi/lo split of pwT (residuals need no scaling:
                # bf16 keeps fp32's exponent range)
                pwh = [phA.tile([128, H], BF16, name=f"pwh{d}", tag=f"pwh{d}")
                       for d in range(DT)]
                pwl = [phA.tile([128, H], BF16, name=f"pwl{d}", tag=f"pwl{d}")
                       for d in range(DT)]
                for d in range(DT):
                    nc.scalar.copy(out=pwh[d], in_=pwT[d])
                    ptmp = ldtmp.tile([128, H], F32, tag="ptmp", bufs=1)
                    nc.vector.tensor_sub(out=ptmp, in0=pwT[d],
                                         in1=pwh[d])
                    nc.scalar.copy(out=pwl[d], in_=ptmp)

                # LayerNorm + transpose to xnT (d, tok)
                xnT = [phA.tile([128, L], F32, name=f"xnT{d}", tag=f"xnT{d}")
                       for d in range(DT)]
                for t in range(TT):
                    x_t = ldtmp.tile([128, D], F32, tag="x_t")
                    nc.sync.dma_start(out=x_t, in_=d_x[t * 128:(t + 1) * 128, :])
                    stats = ldtmp.tile([128, 6], F32, tag="stats")
                    nc.vector.bn_stats(out=stats, in_=x_t)
                    mv = ldtmp.tile([128, 2], F32, tag="mv")
                    nc.vector.bn_aggr(out=mv, in_=stats)
                    rstd = ldtmp.tile([128, 1], F32, tag="rstd")
                    nc.scalar.activation(out=rstd, in_=mv[:, 1:2],
                                         func=mybir.ActivationFunctionType.Sqrt,
                                         bias=eps_t, scale=1.0)
                    nc.vector.reciprocal(out=rstd, in_=rstd)
                    xn = ldtmp.tile([128, D], F32, tag="xn")
                    nc.vector.tensor_scalar(
                        out=xn, in0=x_t, scalar1=mv[:, 0:1], scalar2=rstd,
                        op0=mybir.AluOpType.subtract, op1=mybir.AluOpType.mult)
                    nc.vector.tensor_mul(out=xn, in0=xn, in1=lnw_bc)
                    nc.vector.tensor_add(out=xn, in0=xn, in1=lnb_bc)
                    for d in range(DT):
                        ps_tr = psTr.tile([128, 128], F32, tag="ps_tr")
                        nc.tensor.transpose(ps_tr, xn[:, d * 128:(d + 1) * 128],
                                            ident)
                        nc.scalar.copy(out=xnT[d][:, t * 128:(t + 1) * 128],
                                       in_=ps_tr)

                xnh = [phA.tile([128, L], BF16, name=f"xnh{d}", tag=f"xnh{d}")
                       for d in range(DT)]
                xnl = [phA.tile([128, L], BF16, name=f"xnl{d}", tag=f"xnl{d}")
                       for d in range(DT)]
                for d in range(DT):
                    nc.scalar.copy(out=xnh[d], in_=xnT[d])
                    xtmp = ldtmp.tile([128, L], F32, tag="xtmp", bufs=1)
                    nc.vector.tensor_sub(out=xtmp, in0=xnT[d], in1=xnh[d])
                    nc.scalar.copy(out=xnl[d], in_=xtmp)

                # projection t^T[h,tok] = sum_d pwT[d,h].T @ xnT[d,tok] as
                # bf16 3-pass (hi@hi + hi@lo + lo@hi): ~fp32-exact at
                # 3 cyc/row vs 4 for fp32. tok-chunk outer so phase B can
                # start early.
                for tk in range(TOKC):
                    tks = slice(tk * 512, (tk + 1) * 512)
                    for h in range(HT):
                        hsl = slice(h * 128, (h + 1) * 128)
                        ps_t = psA.tile([128, 512], F32, tag="ps_t", bufs=4)
                        for d in range(DT):
                            nc.tensor.matmul(ps_t, lhsT=pwh[d][:, hsl],
                                             rhs=xnh[d][:, tks],
                                             start=(d == 0), stop=False)
                        for d in range(DT):
                            nc.tensor.matmul(ps_t, lhsT=pwh[d][:, hsl],
                                             rhs=xnl[d][:, tks],
                                             start=False, stop=False)
                        for d in range(DT):
                            nc.tensor.matmul(ps_t, lhsT=pwl[d][:, hsl],
                                             rhs=xnh[d][:, tks],
                                             start=False, stop=(d == DT - 1))
                        nc.scalar.copy(out=th[h][:, tk * 512:(tk + 1) * 512],
                                       in_=ps_t)

            # ---------- phase B: cross matmul + per-chunk argmax ----------
            cval = [persist.tile([128, CCH], F32, name=f"cval{t}", tag=f"cval{t}")
                    for t in range(TT)]
            cidx = [persist.tile([128, CCH], U32, name=f"cidx{t}", tag=f"cidx{t}")
                    for t in range(TT)]

            with tc.tile_pool(name="cbf", bufs=2) as cbf_pool, \
                 tc.tile_pool(name="csq", bufs=2) as csq_pool, \
                 tc.tile_pool(name="strips", bufs=6) as strips, \
                 tc.tile_pool(name="psB", bufs=4, space="PSUM") as psB:

                for cc in range(CCH):
                    csl = slice(cc * CW, (cc + 1) * CW)
                    cb_f = []
                    for h in range(HT):
                        t_ = cbf_pool.tile([128, CW], F32R, name=f"cbf{h}",
                                           tag=f"cbf{h}")
                        nc.sync.dma_start(out=t_, in_=d_cb[h * 128:(h + 1) * 128,
                                                           csl])
                        cb_f.append(t_)
                    # bias_cc = 512 - 0.5*||c||^2 (centering keeps it small so
                    # fp32r rounding of the bias row is harmless; constant
                    # shift is argmax-invariant). Square + free-dim reduce
                    # over cbT rows (c on partitions), DRAM bounce to
                    # re-layout as a [1, CW] row.
                    csq_cols = csq_pool.tile([128, CW // 128], F32,
                                             name="csq_cols", tag="csq_cols")
                    for j in range(CW // 128):
                        cbt_t = csq_pool.tile([128, H], F32, name="cbt_t",
                                              tag="cbt_t", bufs=3)
                        nc.sync.dma_start(
                            out=cbt_t,
                            in_=d_cbt[cc * CW + j * 128:cc * CW + (j + 1) * 128, :])
                        sq_t = csq_pool.tile([128, H], F32, name="sq_t",
                                             tag="sq_t", bufs=3)
                        nc.scalar.activation(out=sq_t, in_=cbt_t,
                                             func=mybir.ActivationFunctionType.Square)
                        nc.vector.tensor_reduce(
                            out=csq_cols[:, j:j + 1], in_=sq_t,
                            axis=mybir.AxisListType.X, op=mybir.AluOpType.add)
                    nc.vector.tensor_scalar(
                        out=csq_cols, in0=csq_cols, scalar1=-0.5, scalar2=512.0,
                        op0=mybir.AluOpType.mult, op1=mybir.AluOpType.add)
                    nc.sync.dma_start(
                        out=bass.AP(tensor=scratch.tensor, offset=scratch.offset
                                    + cc * CW, ap=[[1, 128], [128, CW // 128]]),
                        in_=csq_cols)
                    bias_f = csq_pool.tile([1, CW], F32, name="bias_f",
                                           tag="bias_f")
                    nc.sync.dma_start(
                        out=bias_f,
                        in_=bass.AP(tensor=scratch.tensor, offset=scratch.offset
                                    + cc * CW, ap=[[1, 1], [1, CW]]))
                    # bias values span +-150 (chi^2 spread of ||c||^2), so a
                    # single tf32 row quantizes at ~0.1 — enough to flip
                    # argmins. Split hi + residual; K=2 ones matmul streams
                    # the same rows, so the extra precision is free on PE.
                    bias_hi = csq_pool.tile([1, CW], F32R, name="bias_hi",
                                            tag="bias_hi")
                    nc.scalar.copy(out=bias_hi, in_=bias_f)
                    bias_lo = csq_pool.tile([1, CW], F32, name="bias_lo",
                                            tag="bias_lo")
                    nc.vector.tensor_sub(out=bias_lo, in0=bias_f,
                                         in1=bias_hi.bitcast(F32))
                    bias_lo_r = csq_pool.tile([1, CW], F32R, name="bias_lo_r",
                                              tag="bias_lo_r")
                    nc.scalar.copy(out=bias_lo_r, in_=bias_lo)
                    bias2 = csq_pool.tile([2, CW], F32R, name="bias2",
                                          tag="bias2")
                    nc.sync.dma_start(out=bias2[0:1, :], in_=bias_hi)
                    nc.sync.dma_start(out=bias2[1:2, :], in_=bias_lo_r)

                    for t in range(TT):
                        tsl = slice(t * 128, (t + 1) * 128)
                        acc = psB.tile([128, CW], F32, tag="acc")
                        for g in range(NG):
                            gsl = slice(g * 512, (g + 1) * 512)
                            for h in range(HT):
                                nc.tensor.matmul(acc[:, gsl],
                                                 lhsT=th[h][:, tsl],
                                                 rhs=cb_f[h][:, gsl],
                                                 start=(h == 0), stop=False)
                            nc.tensor.matmul(acc[:, gsl], lhsT=ones2,
                                             rhs=bias2[:, gsl],
                                             start=False, stop=True)
                        mx8 = strips.tile([128, 8], F32, tag="mx8")
                        nc.vector.max(out=mx8, in_=acc)
                        ix8 = strips.tile([128, 8], U32, tag="ix8")
                        nc.vector.max_index(out=ix8, in_max=mx8, in_values=acc)
                        nc.gpsimd.tensor_copy(out=cval[t][:, cc:cc + 1],
                                              in_=mx8[:, 0:1])
                        nc.gpsimd.tensor_copy(out=cidx[t][:, cc:cc + 1],
                                              in_=ix8[:, 0:1])

            # ---------- phase C: combine the chunk winners ----------
            with tc.tile_pool(name="fin", bufs=2) as fin:
                for t in range(TT):
                    cidxf = fin.tile([128, CCH], F32, tag="cidxf")
                    nc.vector.tensor_copy(cidxf, cidx[t])
                    gmx = fin.tile([128, 8], F32, tag="gmx")
                    nc.vector.max(out=gmx, in_=cval[t])
                    mask = fin.tile([128, CCH], F32, tag="mask")
                    nc.vector.tensor_scalar(
                        out=mask, in0=cval[t], scalar1=gmx[:, 0:1], scalar2=None,
                        op0=mybir.AluOpType.is_ge)
                    inv = fin.tile([128, CCH], F32, tag="inv")
                    nc.vector.tensor_scalar(
                        out=inv, in0=mask, scalar1=-32768.0, scalar2=32768.0,
                        op0=mybir.AluOpType.mult, op1=mybir.AluOpType.add)
                    cand = fin.tile([128, CCH], F32, tag="cand")
                    nc.vector.tensor_add(cand, cidxf, chunk_off)
                    nc.vector.tensor_add(cand, cand, inv)
                    win = fin.tile([128, 1], F32, tag="win")
                    nc.vector.tensor_reduce(out=win, in_=cand,
                                            axis=mybir.AxisListType.X,
                                            op=mybir.AluOpType.min)
                    lab = fin.tile([128, 1], I32, tag="lab")
                    nc.vector.tensor_copy(lab, win)
                    nc.sync.dma_start(out=d_lab[:, t:t + 1], in_=lab)

    nc.compile()
    return nc


_NC_CACHE = None


def make_in_maps(inputs):
    input_values = np.ascontiguousarray(inputs["input_values"], np.float32)
    pw = np.ascontiguousarray(inputs["proj_weight"], np.float32)
    lnw = np.ascontiguousarray(inputs["ln_weight"], np.float32)
    lnb = np.ascontiguousarray(inputs["ln_bias"], np.float32)
    cb = np.ascontiguousarray(inputs["codebook"], np.float32)
    cbt = np.ascontiguousarray(cb.T)

    in_maps = []
    for i in range(N_CORES):
        in_maps.append({
            "x": np.ascontiguousarray(input_values[i]),
            "pw": pw, "lnw": lnw, "lnb": lnb, "cb": cb, "cbt": cbt,
        })
    return in_maps


def kernel(input_values, ln_weight, ln_bias, proj_weight, codebook):
    global _NC_CACHE
    if _NC_CACHE is None:
        _NC_CACHE = build_nc()
    nc = _NC_CACHE

    in_maps = make_in_maps(dict(
        input_values=input_values, ln_weight=ln_weight, ln_bias=ln_bias,
        proj_weight=proj_weight, codebook=codebook))
    res = run_bass_kernel_spmd(nc, in_maps, core_ids=list(range(N_CORES)))
    out = np.empty((B, L), np.int32)
    for i in range(N_CORES):
        out[i] = res.results[i]["labels"].T.reshape(L)
    return out
